# revision 1
# baseline (speedup 1.0000x reference)
"""CycleFC forward on 8 Trainium2 NeuronCores.

Problem: x [64, 256, 56, 56] f32, weight [256, 256], bias [256].
  out[b,o,h,w] = sum_c weight[o,c] * x[b,c,h,w+s_c] + bias[o]
  with s_c = (c+3) % 7 - 3 and zero padding outside [0, W).

Strategy:
  - Data-parallel over batch: 8 batches per core.
  - The per-channel shift is absorbed into the DMA load offset: the host
    pads each (c, h) row to stride 59 ([3 zeros][56 data]; a row's
    right-shift reads land in the next row's left-pad zeros) so channel c's
    whole padded plane is loaded as ONE contiguous run starting at element
    (3 + s_c).  After that, every channel's SBUF row holds
    xs[c, h*59 + w] = x[c, h, w + s_c] (zeros off the edge), so a plain
    matmul with a strided rhs access pattern ([h-rows, 59-stride] x [56, 1])
    computes the shifted 1x1 conv exactly.  Channels are host-permuted so
    that each shift group is a contiguous partition range (weights permuted
    to match along the contraction dim only; output channel order is
    untouched).
  - matmul in float32r (1 cycle/row vs 4 for float32); inputs keep fp32
    bits, PSUM accumulates fp32.  rel err vs fp32 reference ~1.4e-4.
  - Input loads on the SP HWDGE ring, output stores on the ACT HWDGE ring
    (separate FIFOs - stores gated on compute must not head-of-line-block
    the prefetch loads).
"""

import contextlib

import numpy as np

C = 256
H = 56
W = 56
B_PER_CORE = 8
N_CORES = 8
K = 7
WP = 59           # padded row stride ([3 zeros][56 data] per row; row h's
                  # right-pad reads land in row h+1's left-pad zeros)
PLANE = H * WP + (62 - WP)   # DRAM plane: + tail zeros for the max shift
TILE_PLANE = H * WP          # SBUF tile free size (divisible by WP)
LOAD = (H - 1) * WP + W      # elements DMAed per channel (covers max AP read)
HW = H * W        # 3136
ROWS_PER_MM = 8   # h-rows per matmul -> free dim 448 (<=512 fp32 PSUM bank)
NT = H // ROWS_PER_MM  # 7 n-tiles
FREE = ROWS_PER_MM * W  # 448

# shift for channel group j (channels c with c % 7 == j, permuted contiguous)
_SHIFTS = [(j + 3) % K - K // 2 for j in range(K)]          # [0,1,2,3,-3,-2,-1]
_GROUP_SIZES = [len(range(j, C, K)) for j in range(K)]       # [37,37,37,37,36,36,36]
_GROUP_STARTS = np.cumsum([0] + _GROUP_SIZES).tolist()


def _chunk_segments():
    """Per 128-partition contraction chunk: list of (local_lo, local_hi, shift)."""
    segs = [[], []]
    for j in range(K):
        glo, ghi = _GROUP_STARTS[j], _GROUP_STARTS[j + 1]
        for chunk in range(2):
            c0, c1 = chunk * 128, chunk * 128 + 128
            lo, hi = max(glo, c0), min(ghi, c1)
            if lo < hi:
                segs[chunk].append((lo - c0, hi - c0, _SHIFTS[j]))
    return segs


def build_nc(mm_dtype="float32r", x_bufs=4, o_bufs=3, ps_bufs=8,
             store_eng="scalar", reps=1, loop_reps=0, dma_only=0, tiny_loop=0):
    """Build the single-core Bass program (SPMD across 8 cores).

    reps/loop_reps/dma_only/tiny_loop are dev-only knobs for timing probes.
    """
    import concourse.mybir as mybir
    import concourse.tile as tile
    from concourse import bacc

    f32 = mybir.dt.float32
    mmdt = getattr(mybir.dt, mm_dtype)

    nc = bacc.Bacc("TRN2", target_bir_lowering=False, debug=False,
                   enable_asserts=False)
    xp = nc.dram_tensor("xp", [B_PER_CORE, C, PLANE], mmdt,
                        kind="ExternalInput").ap()
    wT = nc.dram_tensor("wT", [C, C], mmdt, kind="ExternalInput").ap()
    biasT = nc.dram_tensor("biasT", [128, 2], f32, kind="ExternalInput").ap()
    out = nc.dram_tensor("out", [B_PER_CORE, C, HW], f32,
                         kind="ExternalOutput").ap()

    segs = _chunk_segments()
    store = getattr(nc, store_eng)

    def one_pass(rep, xpool, opool, pspool, w0, w1, bt):
        for b in range(B_PER_CORE):
            xs = []
            for chunk in range(2):
                xt = xpool.tile([128, TILE_PLANE], mmdt, tag="x",
                                name=f"x_r{rep}b{b}c{chunk}")
                for (lo, hi, s) in segs[chunk]:
                    off = 3 + s
                    nc.sync.dma_start(
                        xt[lo:hi, 0:LOAD],
                        xp[b, chunk * 128 + lo:chunk * 128 + hi,
                           off:off + LOAD])
                xs.append(xt)
            rhs_views = [x[:].rearrange("p (h w) -> p h w", w=WP) for x in xs]
            for o in range(2):
                osb = opool.tile([128, HW], f32, tag="o",
                                 name=f"o_r{rep}b{b}o{o}")
                if dma_only:
                    nc.vector.memset(osb[:, 0:8], 0.0)
                    store.dma_start(out[b, o * 128:(o + 1) * 128, :], osb[:])
                    continue
                for t in range(NT):
                    ps = pspool.tile([128, FREE], f32, tag="ps",
                                     name=f"ps_r{rep}b{b}o{o}t{t}")
                    for chunk in range(2):
                        rhs = rhs_views[chunk][
                            :, t * ROWS_PER_MM:(t + 1) * ROWS_PER_MM, 0:W]
                        lhsT = (w0 if chunk == 0 else w1)[
                            :, o * 128:(o + 1) * 128]
                        nc.tensor.matmul(ps[:], lhsT, rhs,
                                         start=(chunk == 0), stop=(chunk == 1))
                    nc.vector.tensor_scalar(
                        out=osb[:, t * FREE:(t + 1) * FREE],
                        in0=ps[:],
                        scalar1=bt[:, o:o + 1],
                        scalar2=None,
                        op0=mybir.AluOpType.add)
                store.dma_start(out[b, o * 128:(o + 1) * 128, :], osb[:])

    with tile.TileContext(nc) as tc:
        with (
            tc.tile_pool(name="w", bufs=1) as wpool,
            tc.tile_pool(name="x", bufs=x_bufs) as xpool,
            tc.tile_pool(name="o", bufs=o_bufs) as opool,
            tc.tile_pool(name="ps", bufs=ps_bufs, space="PSUM") as pspool,
        ):
            w0 = wpool.tile([128, C], mmdt, tag="w0")
            w1 = wpool.tile([128, C], mmdt, tag="w1")
            nc.sync.dma_start(w0[:], wT[0:128, :])
            nc.sync.dma_start(w1[:], wT[128:256, :])
            bt = wpool.tile([128, 2], f32, tag="bias")
            nc.sync.dma_start(bt[:], biasT[:])

            loop_cm = tc.For_i(0, loop_reps, 1) if loop_reps else \
                contextlib.nullcontext()
            with loop_cm:
                if tiny_loop:
                    xt = xpool.tile([128, 512], mmdt, tag="x", name="tiny")
                    nc.sync.dma_start(xt[:], xp[0, 0:128, 0:512])
                    store.dma_start(out[0, 0:128, 0:512],
                                    xt[:].bitcast(f32))
                else:
                    for rep in range(reps):
                        one_pass(rep, xpool, opool, pspool, w0, w1, bt)
    nc.compile()
    return nc


def _host_prep(x, weight, bias):
    perm = np.concatenate([np.arange(j, C, K) for j in range(K)])
    xp = np.zeros((x.shape[0], C, PLANE), dtype=np.float32)
    xp[:, :, :H * WP].reshape(x.shape[0], C, H, WP)[:, :, :, 3:3 + W] = x[:, perm]
    wT = np.ascontiguousarray(weight[:, perm].T.astype(np.float32))
    biasT = np.ascontiguousarray(bias.astype(np.float32).reshape(2, 128).T)
    return xp, wT, biasT


_NC_CACHE = {}


def _get_nc(mm_dtype="float32r"):
    if mm_dtype not in _NC_CACHE:
        _NC_CACHE[mm_dtype] = build_nc(mm_dtype)
    return _NC_CACHE[mm_dtype]


def kernel(x, weight, bias, mm_dtype="float32r"):
    from concourse.bass_utils import run_bass_kernel_spmd

    x = np.asarray(x, dtype=np.float32)
    weight = np.asarray(weight, dtype=np.float32)
    bias = np.asarray(bias, dtype=np.float32)
    B = x.shape[0]
    assert B == B_PER_CORE * N_CORES and x.shape[1:] == (C, H, W)

    nc = _get_nc(mm_dtype)
    xp, wT, biasT = _host_prep(x, weight, bias)
    in_maps = [
        {"xp": np.ascontiguousarray(xp[c * B_PER_CORE:(c + 1) * B_PER_CORE]),
         "wT": wT, "biasT": biasT}
        for c in range(N_CORES)
    ]
    res = run_bass_kernel_spmd(nc, in_maps, core_ids=list(range(N_CORES)))
    out = np.concatenate(
        [r["out"].reshape(B_PER_CORE, C, H, W) for r in res.results], axis=0)
    return out



# revision 2
# speedup vs baseline: 2.3871x; 2.3871x over previous
"""CycleFC forward on 8 Trainium2 NeuronCores.

Problem: x [64, 256, 56, 56] f32, weight [256, 256], bias [256].
  out[b,o,h,w] = sum_c weight[o,c] * x[b,c,h,w+s_c] + bias[o]
  with s_c = (c+3) % 7 - 3 and zero padding outside [0, W).

Strategy (DMA-bandwidth bound problem => minimize HBM bytes):
  - Data-parallel over batch: 8 batches per core.
  - The per-channel shift is applied ON THE HOST: each channel's plane is
    padded to row stride 59 ([3 zeros][56 data]; a row's right-shift reads
    land in the next row's left-pad zeros) and then rolled by its shift
    s_c.  Every channel then reads from the SAME dram offset, so a whole
    128-channel contraction chunk loads as ONE 2D DMA ([128, 3301]
    contiguous per channel).  After the load, SBUF holds
    xs[c, h*59 + w] = x[c, h, w + s_c] (zeros off the edge) and a plain
    matmul with a strided rhs access pattern computes the shifted 1x1
    conv exactly.  Channels are host-permuted so shift groups are
    contiguous (weights permuted to match along the contraction dim only;
    output channel order is untouched).
  - x is sent as fp8 E3M4 (1 byte/elem, ~1.2% quantization rms), weights
    as bf16 (lhsT), PSUM accumulates fp32, output stored as bf16 and
    upcast to f32 on the host.  Total HBM traffic per core drops from
    ~53 MB (f32 in/out) to ~20 MB.  Measured end-to-end relative error
    ~1.2e-2 against the f32 reference, under the 2e-2 budget.  Set
    xdt="bfloat16" for the conservative variant (~0.2% error).
  - Input loads on the SP HWDGE ring, output stores on the ACT HWDGE
    ring (separate FIFOs - stores gated on compute must not
    head-of-line-block the prefetch loads).
  - PSUM->SBUF bias-add + bf16 downcast alternates between the ACT
    (activation w/ per-partition bias) and DVE (tensor_scalar) engines so
    neither becomes the bottleneck.
"""

import numpy as np

C = 256
H = 56
W = 56
B_PER_CORE = 8
N_CORES = 8
K = 7
WP = 59           # padded row stride ([3 zeros][56 data] per row; row h's
                  # right-pad reads land in row h+1's left-pad zeros)
PLANE = 3312      # DRAM plane per channel: >= 6 + 56*59 + tail for shifts
OFF = 3           # fixed dram read offset after host-side per-channel roll
TILE_PLANE = H * WP          # SBUF tile free size 3304 (divisible by WP)
LOAD = (H - 1) * WP + W      # 3301 elements DMAed per channel
HW = H * W        # 3136
ROWS_PER_MM = 8   # h-rows per matmul -> free dim 448 (<=512 fp32 PSUM bank)
NT = H // ROWS_PER_MM  # 7 n-tiles
FREE = ROWS_PER_MM * W  # 448

# shift for channel group j (channels c with c % 7 == j, permuted contiguous)
_SHIFTS = [(j + 3) % K - K // 2 for j in range(K)]          # [0,1,2,3,-3,-2,-1]
_GROUP_SIZES = [len(range(j, C, K)) for j in range(K)]       # [37,37,37,37,36,36,36]
_GROUP_STARTS = np.cumsum([0] + _GROUP_SIZES).tolist()


def build_nc(xdt="float8e3", x_bufs=4, o_bufs=3, ps_bufs=8):
    """Build the single-core Bass program (SPMD across 8 cores)."""
    import concourse.mybir as mybir
    import concourse.tile as tile
    from concourse import bacc

    f32 = mybir.dt.float32
    bf16 = mybir.dt.bfloat16
    xdt_m = getattr(mybir.dt, xdt)

    nc = bacc.Bacc("TRN2", target_bir_lowering=False, debug=False,
                   enable_asserts=False)
    xq = nc.dram_tensor("xq", [B_PER_CORE, C, PLANE], xdt_m,
                        kind="ExternalInput").ap()
    wT = nc.dram_tensor("wT", [C, C], bf16, kind="ExternalInput").ap()
    biasT = nc.dram_tensor("biasT", [128, 2], f32, kind="ExternalInput").ap()
    out = nc.dram_tensor("out", [B_PER_CORE, C, HW], bf16,
                         kind="ExternalOutput").ap()

    with tile.TileContext(nc) as tc:
        with (
            tc.tile_pool(name="w", bufs=1) as wpool,
            tc.tile_pool(name="x", bufs=x_bufs) as xpool,
            tc.tile_pool(name="o", bufs=o_bufs) as opool,
            tc.tile_pool(name="ps", bufs=ps_bufs, space="PSUM") as pspool,
        ):
            w0 = wpool.tile([128, C], bf16, tag="w0")
            w1 = wpool.tile([128, C], bf16, tag="w1")
            nc.sync.dma_start(w0[:], wT[0:128, :])
            nc.sync.dma_start(w1[:], wT[128:256, :])
            bt = wpool.tile([128, 2], f32, tag="bias")
            nc.sync.dma_start(bt[:], biasT[:])

            for b in range(B_PER_CORE):
                xs = []
                for chunk in range(2):
                    xt = xpool.tile([128, TILE_PLANE], xdt_m, tag="x",
                                    name=f"x_b{b}c{chunk}")
                    nc.sync.dma_start(
                        xt[:, 0:LOAD],
                        xq[b, chunk * 128:(chunk + 1) * 128, OFF:OFF + LOAD])
                    xs.append(xt)
                rhs_views = [x[:].rearrange("p (h w) -> p h w", w=WP)
                             for x in xs]
                for o in range(2):
                    osb = opool.tile([128, HW], bf16, tag="o",
                                     name=f"o_b{b}o{o}")
                    for t in range(NT):
                        ps = pspool.tile([128, FREE], f32, tag="ps",
                                         name=f"ps_b{b}o{o}t{t}")
                        for chunk in range(2):
                            rhs = rhs_views[chunk][
                                :, t * ROWS_PER_MM:(t + 1) * ROWS_PER_MM, 0:W]
                            lhsT = (w0 if chunk == 0 else w1)[
                                :, o * 128:(o + 1) * 128]
                            nc.tensor.matmul(ps[:], lhsT, rhs,
                                             start=(chunk == 0),
                                             stop=(chunk == 1))
                        dst = osb[:, t * FREE:(t + 1) * FREE]
                        if (o * NT + t) % 2 == 0:
                            nc.scalar.add(dst, ps[:], bt[:, o:o + 1])
                        else:
                            nc.vector.tensor_scalar(
                                out=dst, in0=ps[:],
                                scalar1=bt[:, o:o + 1], scalar2=None,
                                op0=mybir.AluOpType.add)
                    nc.scalar.dma_start(out[b, o * 128:(o + 1) * 128, :],
                                        osb[:])
    nc.compile()
    return nc


def _host_prep(x, weight, bias, xdt):
    import ml_dtypes

    np_xdt = {"float8e3": ml_dtypes.float8_e3m4,
              "bfloat16": ml_dtypes.bfloat16}[xdt]
    perm = np.concatenate([np.arange(j, C, K) for j in range(K)])
    B = x.shape[0]
    xq = np.zeros((B, C, PLANE), dtype=np_xdt)
    xperm = x[:, perm].astype(np_xdt)
    for j in range(K):
        s = _SHIFTS[j]
        glo, ghi = _GROUP_STARTS[j], _GROUP_STARTS[j + 1]
        lo = OFF - s
        dst = xq[:, glo:ghi, lo:lo + H * WP]
        dst.reshape(B, ghi - glo, H, WP)[:, :, :, :W] = xperm[:, glo:ghi]
    wT = np.ascontiguousarray(
        weight[:, perm].T.astype(ml_dtypes.bfloat16))
    biasT = np.ascontiguousarray(bias.astype(np.float32).reshape(2, 128).T)
    return xq, wT, biasT


_NC_CACHE = {}


def _get_nc(xdt="float8e3"):
    if xdt not in _NC_CACHE:
        _NC_CACHE[xdt] = build_nc(xdt)
    return _NC_CACHE[xdt]


def kernel(x, weight, bias, mm_dtype="float8e3"):
    from concourse.bass_utils import run_bass_kernel_spmd

    x = np.asarray(x, dtype=np.float32)
    weight = np.asarray(weight, dtype=np.float32)
    bias = np.asarray(bias, dtype=np.float32)
    B = x.shape[0]
    assert B == B_PER_CORE * N_CORES and x.shape[1:] == (C, H, W)

    nc = _get_nc(mm_dtype)
    xq, wT, biasT = _host_prep(x, weight, bias, mm_dtype)
    in_maps = [
        {"xq": np.ascontiguousarray(xq[c * B_PER_CORE:(c + 1) * B_PER_CORE]),
         "wT": wT, "biasT": biasT}
        for c in range(N_CORES)
    ]
    res = run_bass_kernel_spmd(nc, in_maps, core_ids=list(range(N_CORES)))
    out = np.concatenate(
        [np.asarray(r["out"]).astype(np.float32).reshape(B_PER_CORE, C, H, W)
         for r in res.results], axis=0)
    return out


# revision 28
# speedup vs baseline: 2.9477x; 1.2348x over previous
"""CycleFC forward on 8 Trainium2 NeuronCores.

Problem: x [64, 256, 56, 56] f32, weight [256, 256], bias [256].
  out[b,o,h,w] = sum_c weight[o,c] * x[b,c,h,w+s_c] + bias[o]
  with s_c = (c+3) % 7 - 3 and zero padding outside [0, W).

Strategy (DMA-bandwidth bound problem => minimize HBM bytes):
  - Data-parallel over batch: 8 batches per core.
  - The per-channel shift is applied ON THE HOST: each channel's plane is
    padded to row stride 59 ([3 zeros][56 data]; a row's right-shift reads
    land in the next row's left-pad zeros) and then rolled by its shift
    s_c.  Every channel then reads from the SAME dram offset, so a whole
    128-channel contraction chunk loads as ONE 2D DMA ([128, 3301]
    contiguous per channel).  After the load, SBUF holds
    xs[c, h*59 + w] = x[c, h, w + s_c] (zeros off the edge) and a plain
    matmul with a strided rhs access pattern computes the shifted 1x1
    conv exactly.  Channels are host-permuted so shift groups are
    contiguous (weights permuted to match along the contraction dim only;
    output channel order is untouched).
  - x is sent as fp8 E3M4 (1 byte/elem, ~1.2% quantization rms), weights
    as bf16 (lhsT), PSUM accumulates fp32, output stored as bf16 and
    upcast to f32 on the host.  Total HBM traffic per core drops from
    ~53 MB (f32 in/out) to ~20 MB.  Measured end-to-end relative error
    ~1.2e-2 against the f32 reference, under the 2e-2 budget.  Set
    xdt="bfloat16" for the conservative variant (~0.2% error).
  - Input loads on the SP HWDGE ring, output stores on the ACT HWDGE
    ring (separate FIFOs - stores gated on compute must not
    head-of-line-block the prefetch loads).
  - PSUM->SBUF bias-add + bf16 downcast alternates between the ACT
    (activation w/ per-partition bias) and DVE (tensor_scalar) engines so
    neither becomes the bottleneck.
"""

import numpy as np

C = 256
H = 56
W = 56
B_PER_CORE = 8
N_CORES = 8
K = 7
WP = 59           # padded row stride ([3 zeros][56 data] per row; row h's
                  # right-pad reads land in row h+1's left-pad zeros)
PLANE = 3312      # DRAM plane per channel: >= 6 + 56*59 + tail for shifts
OFF = 3           # fixed dram read offset after host-side per-channel roll
TILE_PLANE = H * WP          # SBUF tile free size 3304 (divisible by WP)
LOAD = (H - 1) * WP + W      # 3301 elements DMAed per channel
HW = H * W        # 3136
ROWS_PER_MM = 8   # h-rows per matmul -> free dim 448 (<=512 fp32 PSUM bank)
NT = H // ROWS_PER_MM  # 7 n-tiles
FREE = ROWS_PER_MM * W  # 448

# shift for channel group j (channels c with c % 7 == j, permuted contiguous)
_SHIFTS = [(j + 3) % K - K // 2 for j in range(K)]          # [0,1,2,3,-3,-2,-1]
_GROUP_SIZES = [len(range(j, C, K)) for j in range(K)]       # [37,37,37,37,36,36,36]
_GROUP_STARTS = np.cumsum([0] + _GROUP_SIZES).tolist()


def build_nc(xdt="float8e3", x_bufs=7, o_bufs=4, ps_bufs=8,
             head_split=True, tail_split=4, store_eng="gpsimd"):
    """Build the single-core Bass program (SPMD across 8 cores).

    tail_split: the last batch's two column stores are emitted in pieces of
    `tail_split` t-tiles so the final transfers chase the bias-adds instead
    of waiting for the full 7-tile column (0 = single store).
    """
    import concourse.mybir as mybir
    import concourse.tile as tile
    from concourse import bacc

    f32 = mybir.dt.float32
    bf16 = mybir.dt.bfloat16
    xdt_m = getattr(mybir.dt, xdt)

    nc = bacc.Bacc("TRN2", target_bir_lowering=False, debug=False,
                   enable_asserts=False)
    xq = nc.dram_tensor("xq", [B_PER_CORE, C, PLANE], xdt_m,
                        kind="ExternalInput").ap()
    # both 128-row contraction chunks side by side, plus three (bf16) bias
    # columns in the tail: [bias_o0, bias_o1, 4*bias_o1] - one DMA for all
    wT = nc.dram_tensor("wT", [128, 2 * C + 3], bf16,
                        kind="ExternalInput").ap()
    # o=0 output chunks leave as bf16; o=1 chunks leave as fp8 E3M4 scaled
    # by 4 (host divides back).  This halves half the store traffic while
    # keeping the end-to-end error at ~1.64e-2 (fp8 quantization of half
    # the output adds ~0.9e-2 in quadrature to the x-quantization 1.36e-2).
    out_bf = nc.dram_tensor("out_bf", [B_PER_CORE, 128, HW], bf16,
                            kind="ExternalOutput").ap()
    out_f8 = nc.dram_tensor("out_f8", [B_PER_CORE, 128, HW],
                            mybir.dt.float8e3, kind="ExternalOutput").ap()

    with tile.TileContext(nc) as tc:
        with (
            tc.tile_pool(name="w", bufs=1) as wpool,
            tc.tile_pool(name="x", bufs=x_bufs) as xpool,
            tc.tile_pool(name="o", bufs=o_bufs) as opool,
            tc.tile_pool(name="ps", bufs=ps_bufs, space="PSUM") as pspool,
        ):
            w01 = wpool.tile([128, 2 * C + 3], bf16, tag="w01")
            bt = wpool.tile([128, 3], f32, tag="bias")
            # weights+bias ride the SP ring ahead of the x loads
            nc.sync.dma_start(w01[:], wT[:, :])
            # one-time upcast: DVE's tensor_scalar needs an f32 scalar operand
            nc.scalar.copy(bt[:], w01[:, 2 * C:2 * C + 3])

            HEAD = 2 * ROWS_PER_MM * WP  # 944 elems: rows for t0+t1

            for b in range(B_PER_CORE):
                xs = []
                for chunk in range(2):
                    xt = xpool.tile([128, TILE_PLANE], xdt_m, tag="x",
                                    name=f"x_b{b}c{chunk}")
                    xs.append(xt)
                if b == 0 and head_split:
                    # split batch 0's loads: the two heads (rows for t0/t1)
                    # land first so the PE starts earlier; the tails stream
                    # behind.  Region-granular tile deps let the t0/t1
                    # matmuls fire off the heads alone.
                    for chunk in range(2):
                        nc.sync.dma_start(
                            xs[chunk][:, 0:HEAD],
                            xq[b, chunk * 128:(chunk + 1) * 128,
                               OFF:OFF + HEAD])
                    for chunk in range(2):
                        nc.sync.dma_start(
                            xs[chunk][:, HEAD:LOAD],
                            xq[b, chunk * 128:(chunk + 1) * 128,
                               OFF + HEAD:OFF + LOAD])
                else:
                    for chunk in range(2):
                        nc.sync.dma_start(
                            xs[chunk][:, 0:LOAD],
                            xq[b, chunk * 128:(chunk + 1) * 128,
                               OFF:OFF + LOAD])
                rhs_views = [x[:].rearrange("p (h w) -> p h w", w=WP)
                             for x in xs]
                last = b == B_PER_CORE - 1
                split = tail_split if (last or b == 0) else 0
                for o in range(2):
                    f8 = o == 1
                    odst = out_f8 if f8 else out_bf
                    osb = opool.tile([128, HW], mybir.dt.float8e3 if f8
                                     else bf16, tag="o8" if f8 else "o",
                                     name=f"o_b{b}o{o}")
                    done_t = 0
                    for t in range(NT):
                        ps = pspool.tile([128, FREE], f32, tag="ps",
                                         name=f"ps_b{b}o{o}t{t}")
                        for chunk in range(2):
                            rhs = rhs_views[chunk][
                                :, t * ROWS_PER_MM:(t + 1) * ROWS_PER_MM, 0:W]
                            lhsT = w01[:, chunk * C + o * 128:
                                       chunk * C + (o + 1) * 128]
                            nc.tensor.matmul(ps[:], lhsT, rhs,
                                             start=(chunk == 0),
                                             stop=(chunk == 1))
                        dst = osb[:, t * FREE:(t + 1) * FREE]
                        if (o * NT + t) % 2 == 0:
                            if f8:
                                # out = ps*4 + (4*bias)  [bt col 2]
                                nc.scalar.activation(
                                    dst, ps[:],
                                    mybir.ActivationFunctionType.Identity,
                                    bias=bt[:, 2:3], scale=4.0)
                            else:
                                nc.scalar.add(dst, ps[:], bt[:, 0:1])
                        else:
                            if f8:
                                # out = (ps + bias) * 4
                                nc.vector.tensor_scalar(
                                    out=dst, in0=ps[:],
                                    scalar1=bt[:, 1:2], scalar2=4.0,
                                    op0=mybir.AluOpType.add,
                                    op1=mybir.AluOpType.mult)
                            else:
                                nc.vector.tensor_scalar(
                                    out=dst, in0=ps[:],
                                    scalar1=bt[:, 0:1], scalar2=None,
                                    op0=mybir.AluOpType.add)
                        if (split and (t + 1 - done_t >= split
                                       or t == NT - 1)):
                            getattr(nc, store_eng).dma_start(
                                odst[b, :, done_t * FREE:(t + 1) * FREE],
                                osb[:, done_t * FREE:(t + 1) * FREE])
                            done_t = t + 1
                    if not split:
                        getattr(nc, store_eng).dma_start(odst[b, :, :],
                                                         osb[:])
    nc.compile()
    return nc


def _host_prep(x, weight, bias, xdt):
    import ml_dtypes

    np_xdt = {"float8e3": ml_dtypes.float8_e3m4,
              "bfloat16": ml_dtypes.bfloat16}[xdt]
    perm = np.concatenate([np.arange(j, C, K) for j in range(K)])
    B = x.shape[0]
    xq = np.zeros((B, C, PLANE), dtype=np_xdt)
    xperm = x[:, perm].astype(np_xdt)
    for j in range(K):
        s = _SHIFTS[j]
        glo, ghi = _GROUP_STARTS[j], _GROUP_STARTS[j + 1]
        lo = OFF - s
        dst = xq[:, glo:ghi, lo:lo + H * WP]
        dst.reshape(B, ghi - glo, H, WP)[:, :, :, :W] = xperm[:, glo:ghi]
    wTf = weight[:, perm].T.astype(ml_dtypes.bfloat16)   # [C(contr), C_out]
    b2 = bias.reshape(2, 128).T.astype(np.float32)        # [128, 2]
    bcols = np.concatenate([b2, 4.0 * b2[:, 1:2]], axis=1)
    wT = np.ascontiguousarray(np.concatenate(
        [wTf[0:128, :], wTf[128:256, :],
         bcols.astype(ml_dtypes.bfloat16)], axis=1))
    return xq, wT


_NC_CACHE = {}


def _get_nc(xdt="float8e3"):
    if xdt not in _NC_CACHE:
        _NC_CACHE[xdt] = build_nc(xdt)
    return _NC_CACHE[xdt]


def kernel(x, weight, bias, mm_dtype="float8e3"):
    from concourse.bass_utils import run_bass_kernel_spmd

    x = np.asarray(x, dtype=np.float32)
    weight = np.asarray(weight, dtype=np.float32)
    bias = np.asarray(bias, dtype=np.float32)
    B = x.shape[0]
    assert B == B_PER_CORE * N_CORES and x.shape[1:] == (C, H, W)

    nc = _get_nc(mm_dtype)
    xq, wT = _host_prep(x, weight, bias, mm_dtype)
    in_maps = [
        {"xq": np.ascontiguousarray(xq[c * B_PER_CORE:(c + 1) * B_PER_CORE]),
         "wT": wT}
        for c in range(N_CORES)
    ]
    res = run_bass_kernel_spmd(nc, in_maps, core_ids=list(range(N_CORES)))
    out = np.empty((B, C, H, W), dtype=np.float32)
    for c, r in enumerate(res.results):
        blk = out[c * B_PER_CORE:(c + 1) * B_PER_CORE]
        blk[:, 0:128] = np.asarray(r["out_bf"]).astype(
            np.float32).reshape(B_PER_CORE, 128, H, W)
        blk[:, 128:256] = np.asarray(r["out_f8"]).astype(
            np.float32).reshape(B_PER_CORE, 128, H, W) * 0.25
    return out
